# revision 38
# baseline (speedup 1.0000x reference)
"""Trainium2 Bass kernel for relative-position attention (nn_Attention).

Reference computation (B=16, C=128, H=W=32, HEADS=4, d=32, N=1024):
    qkv  = W_qkv @ x                          (1x1 conv, per-pixel matmul)
    S    = scale * (q^T k + q^T r)            where r = rw + rh  (broadcast)
         = scale * q^T (k + r)                <- position term folds into k
    P    = softmax(S, axis=-1)
    out  = P @ v^T
Sharding: data-parallel over batch, 2 batches per core on 8 cores.

Design: the kernel is ScalarE-bound -- exp of the full [N,N] score matrix
per (batch, head) is 8.4M elements/core and exp runs ONLY on the ACT
engine at 1 elem/cycle/lane (~73us/core). Everything else is organized to
hide under the exp stream:

  - All matmul operands are bf16 (1 cycle/col vs 2 for f32r, FWL-eligible
    weight loads; ~6e-3 end-to-end rel err vs the 2e-2 gate).
  - S^T chunks are 4x ROW-TILED on the PE (K=d=32 -> four 32-row tiles,
    one per head, run concurrently) so PE time stays under the exp budget
    even with the HAM clock gate cold.
  - Per round (jc, nf): 4 S matmuls -> 2 psum tiles sA (heads 0,1) / sB
    (heads 2,3), single-buffered; exp as TWO [128,1024] activates so the
    next round's first S pair + the previous round's O matmuls run during
    the second activate.
  - O: standard M=128 matmuls; the two heads of a pair land in disjoint
    partition ranges of ONE accumulation group via the column placement
    of their [1 | v^T]-padded stationary (col tiling at position 64 trips
    the quadrant-3 XBUS hardware bug, so real col tiling is off-limits).
    Z (the softmax denominator) rides along as the ones column -> psum
    partitions 0 / 64.
  - v^T is computed directly (x chunk stationary, W_v moving): no PE
    transposes. ps_v lives in the psO pool so the psS single-buffer
    ping-pong is never perturbed.
  - ScalarE does ONLY exp + the Z copies that fit in unavoidable bubbles
    (batch boundary / tail). All other PSUM evacuation is on the DVE.
  - Normalize per head-pair: wide Z copy (rows 0..64 in one op),
    reciprocal_approx_fast directly on the wide tile (no repartition
    hops), one DRAM bounce, partition-broadcast reads, one full-width
    multiply. Tail chains use both HW DMA queues (sync + scalar).
"""

import numpy as np

B, C, H, W = 16, 128, 32, 32
HEADS = 4
D = C // HEADS          # 32
N = H * W               # 1024
SCALE = float(D) ** -0.5
NCORES = 8
BPC = B // NCORES       # batches per core

import os as _os
KV = _os.environ.get("BASS_KV", "2")


def _build_kernel_v2(nc, tc, tile, mybir, x_ap, wT_ap, rw_ap, rh_ap, out_ap):
    import concourse.bass as bass

    f32 = mybir.dt.float32
    bf16 = mybir.dt.bfloat16
    EXPF = mybir.ActivationFunctionType.Exp

    const = tc.alloc_tile_pool(name="const", bufs=1)
    xpool = tc.alloc_tile_pool(name="xpool", bufs=2)
    qkpool = tc.alloc_tile_pool(name="qkpool", bufs=2)
    epool = tc.alloc_tile_pool(name="epool", bufs=6)
    vtpool = tc.alloc_tile_pool(name="vtpool", bufs=2)
    zpool = tc.alloc_tile_pool(name="zpool", bufs=2)
    ospool = tc.alloc_tile_pool(name="ospool", bufs=2)
    psS = tc.alloc_tile_pool(name="psS", bufs=2, space="PSUM")
    psO = tc.alloc_tile_pool(name="psO", bufs=2, space="PSUM")
    dscratch = tc.alloc_tile_pool(name="dscratch", bufs=2, space="DRAM")

    # --- warmup: load the exp table set while the first DMAs run ---
    with tc.high_priority():
        warm = const.tile([1, 8], f32)
        nc.vector.memset(warm[:], 0.5)
        warm2 = const.tile([1, 8], f32)
        nc.scalar.activation(out=warm2[:], in_=warm[:], func=EXPF)

    # --- input loads: x on the sync HW queue, weights on the scalar HW
    # queue (parallel), with high-priority DVE casts for the startup path
    x_bufs, xr_bufs = [], []
    for b in range(BPC):
        x_bufs.append(xpool.tile([128, N], f32, tag=f"x{b}", name=f"x{b}"))
        xr_bufs.append(xpool.tile([128, N], bf16, tag=f"xr{b}",
                                  name=f"xr{b}"))
    with tc.high_priority():
        for half in range(2):
            nc.sync.dma_start(out=x_bufs[0][:, half * 512:(half + 1) * 512],
                              in_=x_ap[0, :, half * 512:(half + 1) * 512])
        w_s = const.tile([128, 3 * C], f32)
        nc.scalar.dma_start(out=w_s[:], in_=wT_ap[:])
        rw_s = const.tile([128, W], f32)
        nc.scalar.dma_start(out=rw_s[:], in_=rw_ap[:])
        rh_s = const.tile([128, H], f32)
        nc.scalar.dma_start(out=rh_s[:], in_=rh_ap[:])
    for half in range(2):
        nc.sync.dma_start(out=x_bufs[1][:, half * 512:(half + 1) * 512],
                          in_=x_ap[1, :, half * 512:(half + 1) * 512])

    with tc.high_priority():
        w_r = const.tile([128, 3 * C], bf16)
        nc.vector.tensor_copy(out=w_r[:], in_=w_s[:])
        nc.vector.tensor_copy(out=xr_bufs[0][:, 0:512], in_=x_bufs[0][:, 0:512])

        # r[p, y*W + x] = rw[p, x] + rh[p, y] via step-0 free dims
        r_s = const.tile([128, N], f32)
        rw_b = bass.AP(tensor=rw_s.tensor, offset=rw_s.offset,
                       ap=[list(rw_s.ap[0]), [0, H], list(rw_s.ap[1])])
        rh_b = bass.AP(tensor=rh_s.tensor, offset=rh_s.offset,
                       ap=[list(rh_s.ap[0]), list(rh_s.ap[1]), [0, W]])
        nc.vector.tensor_add(
            out=r_s[:].rearrange("p (y x) -> p y x", y=H), in0=rh_b, in1=rw_b
        )
    nc.vector.tensor_copy(out=xr_bufs[0][:, 512:1024],
                          in_=x_bufs[0][:, 512:1024])

    # per-PAIR reciprocal-broadcast tiles; one tile per pair index so a
    # batch's deferred multiply is always emitted before the next batch's
    # broadcast overwrites it (Tile deps are emission-ordered)
    rb_tiles = [const.tile([128, N], f32, name=f"rb{p}") for p in range(2)]
    ones_f = const.tile([128, 32], f32)
    zeros_f = const.tile([128, 64], f32)

    # O stationary tiles (one per batch). Layout: [128j, jc, h, 128m];
    # head h holds [1 | v_h^T] at m-columns [64*(h%2), 64*(h%2)+33),
    # zeros elsewhere. The zero/ones fills never change; emitted off the
    # critical path (after batch 0's evacuations).
    vt_tiles = [vtpool.tile([128, 8, HEADS, 128], bf16, tag=f"vt{i}",
                            name=f"vt{i}") for i in range(BPC)]

    def fill_vt(i):
        # DVE copies; gated behind the zeros/ones memsets, which are
        # emitted after the critical-path evacuations (gpsimd is far too
        # slow for bulk copies and thrashes the DVE via SBUF contention)
        vt = vt_tiles[i]
        for col in range(2):
            zb = bass.AP(tensor=zeros_f.tensor, offset=zeros_f.offset,
                         ap=[list(zeros_f.ap[0]), [0, 8], [0, HEADS],
                             [1, 64]])
            nc.vector.tensor_copy(out=vt[:, :, :, 64 * col:64 * (col + 1)],
                                  in_=zb)
        for h in range(HEADS):
            nc.vector.tensor_copy(
                out=vt[:, :, h, 64 * (h % 2):64 * (h % 2) + 1],
                in_=ones_f[:, 0:8].rearrange("p (j o) -> p j o", o=1),
            )

    # ---------------- per-batch pieces ----------------
    def phase_a_qk(b, q_all, kp_all):
        """qkv q/k projection + evacuation, in nf halves so the first S
        round unblocks as soon as half 0 is evacuated."""
        xr = xr_bufs[b]
        ps_q = psS.tile([128, N], f32, tag="s", name=f"ps_q{b}")
        ps_k = psS.tile([128, N], f32, tag="s", name=f"ps_k{b}")
        for half in range(2):
            sl = slice(half * 512, (half + 1) * 512)
            import contextlib
            prio = tc.high_priority() if half == 0 else contextlib.nullcontext()
            with prio:
                nc.tensor.matmul(ps_q[:, sl], lhsT=w_r[:, 0:128],
                                 rhs=xr[:, sl], start=True, stop=True)
                nc.tensor.matmul(ps_k[:, sl], lhsT=w_r[:, 128:256],
                                 rhs=xr[:, sl], start=True, stop=True)
                nc.vector.tensor_copy(out=q_all[:, sl], in_=ps_q[:, sl])
                nc.vector.tensor_add(out=kp_all[:, sl], in0=ps_k[:, sl],
                                     in1=r_s[:, sl])

    def phase_a_v(b, vt_all):
        """v^T computed directly: x chunk stationary, W_v moving. Batch 0
        uses a psO tile (pool is free then); later batches use TWO psS
        half-tiles (parity-preserving) so ps_v never waits on the previous
        batch's normalize to release the psO banks. One accumulation group
        per bank -- a second start=True in the same bank would re-mark the
        full 2KB zero region and wipe earlier chunks."""
        xr = xr_bufs[b]
        if b == 0:
            ps_tiles = [psO.tile([128, N], f32, tag="o", name=f"ps_v{b}")]
            views = [(ps_tiles[0], 0, 8)]
        else:
            ps_tiles = [psS.tile([128, 512], f32, tag="s",
                                 name=f"ps_v{b}_{hf}") for hf in range(2)]
            views = [(ps_tiles[0], 0, 4), (ps_tiles[1], 4, 8)]
        for ps_v, j0, j1 in views:
            for j in range(j1 - j0):
                jc = j0 + j
                bank_j = jc if b == 0 else j
                nc.tensor.matmul(ps_v[:, bank_j * 128:(bank_j + 1) * 128],
                                 lhsT=xr[:, jc * 128:(jc + 1) * 128],
                                 rhs=w_r[:, 256:384],
                                 start=(j % 4 == 0), stop=(j % 4 == 3))
            # vt[p, jc, h, 64*(h%2)+1 : +33] = v^T chunk, explicit APs
            # (Tile slicing drops the inner offset for int-index + >= 64)
            nj = j1 - j0
            for h in range(HEADS):
                c0 = 64 * (h % 2) + 1
                o_ap = bass.AP(
                    tensor=vt_all.tensor,
                    offset=vt_all.offset + j0 * HEADS * 128 + h * 128 + c0,
                    ap=[list(vt_all.ap[0]), [HEADS * 128, nj], [1, D]],
                )
                i_ap = bass.AP(
                    tensor=ps_v.tensor,
                    offset=ps_v.offset + h * D,
                    ap=[list(ps_v.ap[0]), [HEADS * D, nj], [1, D]],
                )
                nc.vector.tensor_copy(out=o_ap, in_=i_ap)

    def make_o_thunks(b, e_tiles, vt_all, po, jc, nf):
        """O matmuls for round (jc, nf): standard M=128 matmuls; the two
        heads of a pair land in disjoint partition ranges of one
        accumulation group via the column placement of their stationary."""
        def run(pair):
            eA_or_B = e_tiles[pair]
            for e in range(2):
                h = 2 * pair + e
                nc.tensor.matmul(
                    po[pair][:, nf * 512:(nf + 1) * 512],
                    lhsT=vt_all[:, jc, h, :],
                    rhs=eA_or_B[:, e * 512:(e + 1) * 512],
                    start=(jc == 0 and e == 0), stop=(jc == 7 and e == 1),
                )
        return run

    def normalize_pair(b, pair, po_p, os_p, last):
        """Z rows live at psum partitions 0 (head 2p) and 64 (head 2p+1).
        Wide Z copy on ScalarE (fits the boundary/tail bubbles), fast
        approximate reciprocal directly on the wide tile, one DRAM bounce,
        broadcast back, one full-width multiply + per-head out DMA."""
        z66 = zpool.tile([65, N], f32, tag="z66", name=f"z66_{b}_{pair}")
        if pair == 0:
            nc.scalar.copy(out=z66[:], in_=po_p[0:65, :])
        else:
            # pair 1's Z copy on the DVE: runs concurrently with pair 0's
            # ScalarE copy in the boundary/tail bubble
            nc.vector.tensor_copy(out=z66[:], in_=po_p[0:65, :])
        def rest(z66=z66, po_p=po_p, os_p=os_p, pair=pair, b=b, last=last):
            return _normalize_rest(b, pair, po_p, os_p, z66, last)
        return rest

    def _normalize_rest(b, pair, po_p, os_p, z66, last):
        rz = zpool.tile([65, N], f32, tag="rz", name=f"rz_{b}_{pair}")
        nc.vector.reciprocal_approx_fast(out=rz[:], in_=z66[:])
        r_d = dscratch.tile([2, N], f32, tag="rd", name=f"rd_{b}_{pair}")
        dq0 = nc.scalar if (last and pair == 1) else nc.sync
        # rows {0, 64} of rz in one DMA via a partition-strided AP
        pstep = list(rz.ap[0])[0]   # per-partition step of the tile's AP
        rz_rows = bass.AP(tensor=rz.tensor, offset=rz.offset,
                          ap=[[64 * pstep, 2], [1, N]])
        dq0.dma_start(out=r_d[:], in_=rz_rows)
        # broadcast into full 64-row blocks: pad rows get harmless
        # duplicates, so no memset is needed for the wide multiply
        rb = rb_tiles[pair]
        dq0.dma_start(out=rb[0:64, :], in_=r_d[0, :].partition_broadcast(64))
        dq0.dma_start(out=rb[64:128, :],
                      in_=r_d[1, :].partition_broadcast(64))

        def tail():
            nc.vector.tensor_mul(out=os_p[:], in0=po_p[:], in1=rb[:])
            for e in range(2):
                h = 2 * pair + e
                dq0.dma_start(
                    out=out_ap[b, h * D:(h + 1) * D, :],
                    in_=os_p[64 * e + 1:64 * e + 1 + D, :],
                )
        return tail

    # ---------------- main schedule ----------------
    prev_o = []
    norm_rests = []
    norm_tails = []

    for b in range(BPC):
        q_all = qkpool.tile([128, N], bf16, tag="q", name=f"q{b}")
        kp_all = qkpool.tile([128, N], bf16, tag="kp", name=f"kp{b}")
        vt_all = vt_tiles[b]
        phase_a_qk(b, q_all, kp_all)
        if b == 0:
            # memsets here so the fills (which depend on them) can't be
            # front-run by the scheduler ahead of the startup evacuations
            nc.vector.memset(ones_f[:], 1.0)
            nc.vector.memset(zeros_f[:], 0.0)
            fill_vt(0)
            for half in range(2):
                sl = slice(half * 512, (half + 1) * 512)
                nc.vector.tensor_copy(out=xr_bufs[1][:, sl],
                                      in_=x_bufs[1][:, sl])

        po = []  # allocated at t==0 (b0) / t==2 (b>0), after phase_a_v

        e_hist = {}
        for t in range(16):
            jc, nf = t // 2, t % 2
            sA = psS.tile([128, N], f32, tag="s", name=f"sA{b}_{t}")
            sB = psS.tile([128, N], f32, tag="s", name=f"sB{b}_{t}")
            for h in (0, 1):
                nc.tensor.matmul(
                    sA[:, (h % 2) * 512:((h % 2) + 1) * 512],
                    lhsT=kp_all[32 * h:32 * h + 32, jc * 128:(jc + 1) * 128],
                    rhs=q_all[32 * h:32 * h + 32, nf * 512:(nf + 1) * 512],
                    start=True, stop=True, tile_position=(32 * h, 0),
                )
            for h in (2, 3):
                nc.tensor.matmul(
                    sB[:, (h % 2) * 512:((h % 2) + 1) * 512],
                    lhsT=kp_all[32 * h:32 * h + 32, jc * 128:(jc + 1) * 128],
                    rhs=q_all[32 * h:32 * h + 32, nf * 512:(nf + 1) * 512],
                    start=True, stop=True, tile_position=(32 * h, 0),
                )
            # phase_a_v + po allocation: t==0 for b0 (psO is free), t==2
            # for later batches (by then the previous batch's normalize
            # multiplies have released the po banks, so a stalled ps_v
            # can't sit in the PE queue ahead of this round's S matmuls).
            # The previous batch's normalize tails are emitted first: Tile
            # deps are emission-ordered and the tail multiply reads the po
            # banks these allocations reuse.
            tv = 0 if b == 0 else 2
            if t == tv:
                while norm_tails:
                    norm_tails.pop(0)()
                phase_a_v(b, vt_all)
                po.extend(psO.tile([128, N], f32, tag="o", name=f"po{b}_{p}")
                          for p in range(2))
            if b > 0 and t == 1:
                while norm_rests:
                    norm_tails.append(norm_rests.pop(0)())
            if b == 0 and t == 2:
                fill_vt(1)

            if b == 0 or t >= 3:
                for _ in range(4):
                    if prev_o:
                        prev_o.pop(0)()

            eA = epool.tile([128, N], bf16, tag="e", name=f"eA{b}_{t}")
            eB = epool.tile([128, N], bf16, tag="e", name=f"eB{b}_{t}")
            nc.scalar.activation(out=eA[:], in_=sA[:], func=EXPF)
            nc.scalar.activation(out=eB[:], in_=sB[:], func=EXPF)
            e_hist[t] = (eA, eB)

            run = make_o_thunks(b, e_hist[t], vt_all, po, jc, nf)
            prev_o.extend([lambda pair=p, run=run: run(pair) for p in range(2)])

        while prev_o:
            prev_o.pop(0)()

        os_tiles = [ospool.tile([128, N], f32, tag="os", name=f"os{b}_{p}")
                    for p in range(2)]
        last = b == BPC - 1
        for p in range(2):
            # z copy emitted now (fills the boundary bubble); the rest of
            # the chain is deferred past the next batch's evacuations so
            # the static DVE schedule can't put it ahead of them
            rest = normalize_pair(b, p, po[p], os_tiles[p], last)
            if last:
                norm_tails.append(rest())
            else:
                norm_rests.append(rest)

    while norm_tails:
        norm_tails.pop(0)()

    for p in (dscratch, psO, psS, ospool, zpool, vtpool, epool,
              qkpool, xpool, const):
        p.release()


def _build_kernel_v1(nc, tc, tile, mybir, x_ap, wT_ap, rw_ap, rh_ap, out_ap):
    import kernel_v1_backup as kv1
    kv1._build_kernel(nc, tc, tile, mybir, x_ap, wT_ap, rw_ap, rh_ap, out_ap)


def build_nc():
    """Build the Bass module (shared by kernel() and test harnesses)."""
    import concourse.bacc as bacc
    import concourse.tile as tile
    from concourse import mybir

    f32 = mybir.dt.float32
    nc = bacc.Bacc("TRN2", target_bir_lowering=False, debug=False,
                   num_devices=NCORES)
    x_ap = nc.dram_tensor("x", [BPC, C, N], f32, kind="ExternalInput").ap()
    wT_ap = nc.dram_tensor("wT", [C, 3 * C], f32, kind="ExternalInput").ap()
    rw_ap = nc.dram_tensor("rw2", [HEADS * D, W], f32, kind="ExternalInput").ap()
    rh_ap = nc.dram_tensor("rh2", [HEADS * D, H], f32, kind="ExternalInput").ap()
    out_ap = nc.dram_tensor("out", [BPC, C, N], f32, kind="ExternalOutput").ap()

    with tile.TileContext(nc) as tc:
        if KV == "1":
            _build_kernel_v1(nc, tc, tile, mybir, x_ap, wT_ap, rw_ap, rh_ap,
                             out_ap)
        else:
            _build_kernel_v2(nc, tc, tile, mybir, x_ap, wT_ap, rw_ap, rh_ap,
                             out_ap)
    nc.compile()
    return nc


def make_in_maps(x, W_qkv, rw, rh):
    x_ = np.ascontiguousarray(np.asarray(x, np.float32).reshape(B, C, N))
    wT = np.ascontiguousarray(np.asarray(W_qkv, np.float32).T)
    wT[:, 0:C] *= SCALE    # fold the attention score scale into q projection
    rw_ = np.ascontiguousarray(np.asarray(rw, np.float32).reshape(HEADS * D, W))
    rh_ = np.ascontiguousarray(np.asarray(rh, np.float32).reshape(HEADS * D, H))
    return [
        {"x": x_[i * BPC:(i + 1) * BPC], "wT": wT, "rw2": rw_, "rh2": rh_}
        for i in range(NCORES)
    ]


def kernel(x, W_qkv, rw, rh):
    from concourse.bass_utils import run_bass_kernel_spmd

    nc = build_nc()
    in_maps = make_in_maps(x, W_qkv, rw, rh)
    res = None
    for attempt in range(3):
        try:
            res = run_bass_kernel_spmd(nc, in_maps, list(range(NCORES)))
            break
        except Exception:
            # transient device errors usually clear on retry
            if attempt == 2:
                raise
    out = np.concatenate([r["out"] for r in res.results], axis=0)
    return out.reshape(B, C, H, W).astype(np.float32)
